# revision 10
# baseline (speedup 1.0000x reference)
"""AnyPrecisionLinear (4-bit LUT-quantized linear) on 8 TRN2 NeuronCores — v2.

y = x @ dequant(qweight, lut).T + bias
  x (4,2048,4096) fp16, qweight (4,11008,128) int32 bitplanes (MSB-first),
  lut (11008,16) fp16 per-row codebook, bias (11008,) fp16.

Sharding: tensor-parallel on out_features; 1376 rows/core padded to 1408.
x replicated; output computed transposed (o, t) per core; host transposes
and concatenates.

v2 pipeline (vs v1 which ran dequant fully before the GEMM):
  - dequant of o-tile g (128 rows) -> W rows to DRAM -> per-(g,jt) DMA-xbar
    transposes into wt[jt] column slices, so GEMM cells (chunk c, g) become
    ready per-128-output-rows and overlap the remaining dequant.
  - GEMM in (o, t) layout: per cell, PSUM (128o, 512t) accumulates 32
    matmuls (stationary wt slice (128k,128o), moving x^T tile (128k,512t)).
    Bias applied in the ACT epilogue (per-partition bias), no bias matmuls.
  - dequant tree on uint8 bitplanes (host byteswaps qweight): 4 DVE ANDs
    (4x mode), ACT Sign for the LSB bit, 8 level-0 affine selects split
    ACT/GPS/DVE, 7 DVE copy_predicated merges with raw {0,2^s} masks.
  - emission order = timestamp-merged plan (dequant units, x prefetches,
    gemm cells in a 2-sweep staircase) so every engine's static stream is
    in rough execution-time order.
"""

import numpy as np
from contextlib import ExitStack

import concourse.bass as bass
import concourse.mybir as mybir
import concourse.tile as tile
from concourse import bacc
from concourse import bass_utils

P = 128
N_CORES = 8
O_FULL = 11008
O_REAL = O_FULL // N_CORES       # 1376
OT = 11
O_SH = OT * P                    # 1408
IN = 4096
T = 8192
JC = 512                         # dequant k-chunk (in weights)
NCH = IN // JC                   # 8
NJT = IN // P                    # 32
TCH = 512
NTCH = T // TCH                  # 16
QB = IN // 8                     # 512 bytes per plane per row

A = mybir.AluOpType
DT = mybir.dt
AF = mybir.ActivationFunctionType

_cached = {}

# ---- emission-plan cost estimates (us) ----
R0 = 14.0      # first dequant chunk complete
DQC = 6.9      # dequant wall time per (g, chunk)
DQG = DQC * NCH
CELL = 7.1     # PE time per (chunk, g) cell
TRL = 6.0      # transpose lag after a g's last chunk


def _plan(serial=False):
    ready = [R0 + (g + 1) * DQG + TRL for g in range(OT)]
    if serial:
        ready = [R0 + OT * DQG + TRL] * OT
    events = []
    for g in range(OT):
        events.append((R0 + g * DQG - 6.0, 0, 'dqload', g))
        for c in range(NCH):
            events.append((R0 + g * DQG + c * DQC, 1, 'dq', (g, c)))

    # sweep 1: one visit per chunk covering the g's ready at that moment;
    # sweep 2 (descending c): the remaining g's. Consecutive same-chunk
    # visits merge (no x reload).
    visits = []
    done = [0] * NTCH
    t = ready[0]
    for c in range(NTCH):
        G = sum(1 for g in range(OT) if ready[g] <= t + 3.0)
        G = max(1, min(G, OT))
        t = max(t, ready[G - 1])
        visits.append([c, 0, G])
        done[c] = G
        t += G * CELL
    for c in reversed(range(NTCH)):
        if done[c] < OT:
            if visits[-1][0] == c:
                visits[-1][2] = OT
            else:
                visits.append([c, done[c], OT])

    # emission times with a running clock. Trace order defines dataflow: a
    # cell MUST be emitted after the dequant+copy events of its g (ready[g]
    # sits TRL+DQC past the last dq event of g), else Tile records no RAW
    # dep at all and the cell reads garbage.
    t = 0.0
    for (c, glo, ghi) in visits:
        t0 = max(t, ready[glo] - 8.0)
        events.append((t0 - 9.0, 2, 'xload', (c, t0)))
        end = t0
        for g in range(glo, ghi):
            tc_ = max(end, ready[g] + 0.5)
            events.append((tc_, 3, 'cell', (c, g)))
            end = tc_ + CELL
        t = end
    events.sort(key=lambda e: (e[0], e[1]))
    return events


def build_v2(cfg=None):
    cfg = cfg or {}
    nc = bacc.Bacc("TRN2", target_bir_lowering=False, debug=False,
                   num_devices=N_CORES)
    x_d = nc.dram_tensor("x", (T, IN), DT.float16, kind="ExternalInput")
    qw8_d = nc.dram_tensor("qw8", (4, O_SH, QB), DT.uint8, kind="ExternalInput")
    lutx_d = nc.dram_tensor("lutx", (O_SH, 16), DT.float32, kind="ExternalInput")
    bias2_d = nc.dram_tensor("bias2", (P, OT), DT.float32, kind="ExternalInput")
    ob8_d = nc.dram_tensor("ob8", (1, JC), DT.uint8, kind="ExternalInput")
    yt_d = nc.dram_tensor("yt", (O_SH, T), DT.float16, kind="ExternalOutput")

    with tile.TileContext(nc) as tc, ExitStack() as ctx:
        const_pool = ctx.enter_context(tc.tile_pool(name="const", bufs=1))
        wt_pool = ctx.enter_context(tc.tile_pool(name="wt", bufs=1))
        dram = ctx.enter_context(tc.tile_pool(name="dram", bufs=1, space="DRAM"))
        dqp = ctx.enter_context(tc.tile_pool(name="dq", bufs=2))
        xp = ctx.enter_context(tc.tile_pool(name="xp", bufs=2))
        pp = ctx.enter_context(tc.tile_pool(name="pp", bufs=6, space="PSUM"))
        yp = ctx.enter_context(tc.tile_pool(name="yp", bufs=4))

        ob8 = const_pool.tile([P, JC], DT.uint8)
        nc.sync.dma_start(ob8[:], ob8_d.ap().to_broadcast((P, JC)))
        bias2_sb = const_pool.tile([P, OT], DT.float32)
        nc.sync.dma_start(bias2_sb[:], bias2_d[:, :])

        wtall = wt_pool.tile([P, NJT * O_SH], DT.float16, name="wtall")
        wt3 = wtall[:].rearrange("p (j o) -> p j o", o=O_SH)
        w_dram = dram.tile([O_SH, IN], DT.float16, name="w_dram")
        stgp = ctx.enter_context(tc.tile_pool(name="stg", bufs=3))

        state = {}   # g -> (qw8_sb, lutx_sb)
        xts = {}     # c -> [32 tiles]

        def do_dqload(g):
            qw8_sb = dqp.tile([P, 4, QB], DT.uint8, tag="qw8", name="qw8")
            nc.sync.dma_start(qw8_sb[:], qw8_d[:, g * P:(g + 1) * P, :]
                                .rearrange("b o k -> o b k"))
            lutx_sb = dqp.tile([P, 16], DT.float32, tag="lutx", name="lutx")
            nc.sync.dma_start(lutx_sb[:], lutx_d[g * P:(g + 1) * P, :])
            state[g] = (qw8_sb, lutx_sb)

        def do_dq(g, c):
            qw8_sb, lutx_sb = state[g]
            bsl = slice(c * (JC // 8), (c + 1) * (JC // 8))
            a = []
            for i in range(4):
                ai = dqp.tile([P, JC], DT.uint8, tag=f"a{i}", name=f"a{i}")
                src = qw8_sb[:, i, bsl, None].to_broadcast((P, JC // 8, 8))
                nc.vector.tensor_tensor(
                    ai[:].rearrange("p (y s) -> p y s", s=8), src,
                    ob8[:].rearrange("p (y s) -> p y s", s=8), A.bitwise_and)
                a.append(ai)
            b0 = dqp.tile([P, JC], DT.float16, tag="b0", name="b0")
            nc.scalar.activation(b0[:], a[3][:], AF.Sign)
            t_t = []
            for j in range(8):
                tj = dqp.tile([P, JC], DT.float16, tag=f"t{j}", name=f"t{j}")
                le = lutx_sb[:, 2 * j:2 * j + 1]
                d = lutx_sb[:, 2 * j + 1:2 * j + 2]
                if j < 4:
                    nc.scalar.activation(tj[:], b0[:], AF.Identity,
                                         bias=le, scale=d)
                elif j < 7:
                    nc.gpsimd.tensor_scalar(tj[:], b0[:], d, le, A.mult, A.add)
                else:
                    nc.vector.tensor_scalar(tj[:], b0[:], d, le, A.mult, A.add)
                t_t.append(tj)

            def cp(dst, msk, src):
                nc.vector.copy_predicated(dst[:].bitcast(DT.int16), msk[:],
                                          src[:].bitcast(DT.int16))
            for m in range(4):
                cp(t_t[2 * m], a[2], t_t[2 * m + 1])
            for m in range(2):
                cp(t_t[4 * m], a[1], t_t[4 * m + 2])
            cp(t_t[0], a[0], t_t[4])
            nc.sync.dma_start(
                w_dram[g * P:(g + 1) * P, c * JC:(c + 1) * JC], t_t[0][:])
            # DMA-xbar transpose into staging, then ONE DVE copy into wtall
            # column slices. The copy makes downstream matmuls depend on a
            # per-engine semaphore (sound) instead of DMA-lane ticks, whose
            # transitive wait elision is unsound under racing schedules.
            stg = stgp.tile([P, JC], DT.float16, tag="stg", name="stg")
            for r in range(4):
                jt = 4 * c + r
                nc.sync.dma_start_transpose(
                    stg[:, r * P:(r + 1) * P],
                    w_dram[g * P:(g + 1) * P, jt * P:(jt + 1) * P])
            nc.vector.tensor_copy(
                wt3[:, 4 * c:4 * c + 4, g * P:(g + 1) * P],
                stg[:].rearrange("p (r o) -> p r o", o=P))

        def do_xload(c):
            tiles = []
            for jt in range(NJT):
                xtile = xp.tile([P, TCH], DT.float16, tag=f"xt{jt}",
                                name=f"xt{jt}")
                nc.sync.dma_start_transpose(
                    xtile[:], x_d[c * TCH:(c + 1) * TCH, jt * P:(jt + 1) * P])
                tiles.append(xtile)
            xts[c] = tiles

        def do_cell(c, g):
            tiles = xts[c]
            ps = pp.tile([P, TCH], DT.float32, tag="ps", name="ps")
            for jt in range(NJT):
                nc.tensor.matmul(ps[:, :],
                                 wtall[:, jt * O_SH + g * P:
                                       jt * O_SH + (g + 1) * P],
                                 tiles[jt][:, :],
                                 start=(jt == 0), stop=(jt == NJT - 1))
            ysb = yp.tile([P, TCH], DT.float16, tag="y", name="y")
            nc.scalar.activation(ysb[:, :], ps[:, :], AF.Identity,
                                 bias=bias2_sb[:, g:g + 1], scale=1.0)
            nc.sync.dma_start(
                yt_d[g * P:(g + 1) * P, c * TCH:(c + 1) * TCH], ysb[:, :])

        for (_, _, kind, payload) in _plan(serial=cfg.get("serial", False)):
            if kind == 'dqload':
                do_dqload(payload)
            elif kind == 'dq':
                do_dq(*payload)
            elif kind == 'xload':
                do_xload(payload[0])
            else:
                do_cell(*payload)

    nc.compile()
    return nc


def kernel(x, qweight, lut, bias, w_bits, cfg=None, _want_results=False,
           _trace=False):
    assert int(w_bits) == 4
    key = tuple(sorted((cfg or {}).items()))
    if key not in _cached:
        _cached[key] = build(cfg)
    nc = _cached[key]

    x2d = np.ascontiguousarray(np.asarray(x).reshape(T, IN)).astype(
        np.float16, copy=False)
    qw8_full = np.ascontiguousarray(qweight).view(np.int32).byteswap() \
        .view(np.uint8).reshape(4, O_FULL, QB)
    lut32 = np.asarray(lut, dtype=np.float32)
    bias32 = np.asarray(bias, dtype=np.float32)
    ob8 = (np.uint8(1) << (7 - np.arange(JC) % 8).astype(np.uint8)) \
        .reshape(1, JC).copy()

    in_maps = []
    for c in range(N_CORES):
        lo = c * O_REAL
        qw8_c = np.zeros((4, O_SH, QB), np.uint8)
        qw8_c[:, :O_REAL] = qw8_full[:, lo:lo + O_REAL]
        lutx_c = np.zeros((O_SH, 16), np.float32)
        lr = lut32[lo:lo + O_REAL]
        lutx_c[:O_REAL, 0::2] = lr[:, 0::2]
        lutx_c[:O_REAL, 1::2] = lr[:, 1::2] - lr[:, 0::2]
        bias2_c = np.zeros((P, OT), np.float32)
        br = np.zeros(O_SH, np.float32)
        br[:O_REAL] = bias32[lo:lo + O_REAL]
        bias2_c[:, :] = br.reshape(OT, P).T
        in_maps.append({"x": x2d, "qw8": qw8_c, "lutx": lutx_c,
                        "bias2": bias2_c, "ob8": ob8})

    res = bass_utils.run_bass_kernel_spmd(nc, in_maps,
                                          core_ids=list(range(N_CORES)),
                                          trace=_trace)
    y = np.empty((T, O_FULL), np.float16)
    for c in range(N_CORES):
        y[:, c * O_REAL:(c + 1) * O_REAL] = res.results[c]["yt"][:O_REAL].T
    out = y.reshape(4, 2048, O_FULL)
    if _want_results:
        return out, res
    return out


build = build_v2
